# revision 26
# baseline (speedup 1.0000x reference)
"""Multi-head causal attention (B=4, S=2048, D=1024, H=16) on 8 NeuronCores.

Sharding: batch x head-group. Core c handles batch b = c//2 and head group
g = c%2 (8 heads of 64 dims each). Wq/Wk/Wv are column-split per head group
(Megatron column-parallel), Wo is row-split; each core returns a partial
output [S, D] which the host sums over the two head-group cores per batch.

Device kernel (identical SPMD program on all 8 cores, bf16 compute,
fp32 accumulation):
  1. QT/KT = (X @ Wg)^T computed directly in transposed layout
     (dk on partitions) so attention matmuls need no on-device transpose.
     V computed in natural layout [sk, dv] and packed with a ones-column
     per head (denominator trick).
  2. Per head: L^T tiles [sk=128, sq=512] = KT_h^T-slices @ QT_h (K=64),
     exp via ScalarE with the 1/sqrt(dk) folded into the activation scale,
     causal masking on diagonal tiles via a preloaded 0/1 mask multiply,
     then C~^T[65, sq] += Vtilde^T @ A^T accumulated over sk chunks: rows
     0..63 are the unnormalized context^T, row 64 the softmax denominator.
     Normalize with reciprocal + DMA row-broadcast + VectorE multiply.
  3. Output projection: ct pair tiles [128(dv), 2048(sq)] @ Wo rows,
     accumulating all 8 heads (K=128 per matmul), PSUM -> DRAM f32.
"""
import json

import numpy as np
import ml_dtypes

BF16 = ml_dtypes.bfloat16

B, S, D = 4, 2048, 1024
H = 16
DK = 64          # per-head dim
HPG = 8          # heads per group
GW = HPG * DK    # group width = 512
N_CORES = 8

_nc_cache = {}
TUNE = {"apool": 8, "xpool": 12, "every": 8, "pl": 3}


def _apply_compat_patches():
    """This container's walrus rejects instructions carrying more than one
    sem-wait ("Too many sync wait commands"). Split excess waits onto NoOps
    on the same engine, patched into every compile path."""
    import concourse.bass_utils as bass_utils

    if getattr(bass_utils, "_wait_split_patched", False):
        return
    _orig = bass_utils.compile_bir_kernel
    seq = [0]

    def split_bir_waits(bir, limit=1):
        for fn in bir.get("functions", []):
            for bb in fn.get("blocks", []):
                out, changed = [], False
                for ins in bb.get("instructions", []):
                    si = ins.get("sync_info")
                    ow = (si or {}).get("on_wait") or []
                    if len(ow) > limit:
                        changed = True
                        extra, keep = ow[:-limit], ow[-limit:]
                        for i in range(0, len(extra), limit):
                            seq[0] += 1
                            out.append({
                                "debug": ins.get("debug", 0),
                                "engine": ins["engine"],
                                "ins": [], "outs": [],
                                "name": f"WSPLIT-{seq[0]}",
                                "opcode": "NoOp",
                                "sync_info": {"on_update": [],
                                              "on_wait": extra[i:i + limit]},
                            })
                        si["on_wait"] = keep
                    out.append(ins)
                if changed:
                    bb["instructions"] = out
        return bir

    def _patched(bir_json, tmpdir, neff_name="file.neff", **kw):
        bir = split_bir_waits(json.loads(bir_json))
        return _orig(json.dumps(bir).encode(), tmpdir, neff_name, **kw)

    bass_utils.compile_bir_kernel = _patched
    bass_utils._wait_split_patched = True
    try:
        import concourse.bass2jax as bass2jax
        bass2jax.compile_bir_kernel = _patched
    except Exception:
        pass


def build_attention_nc():
    """Build the SPMD Bass program (one NeuronCore's view).

    Emission order interleaves Q-projection with pr=0 attention per head
    pair, and slots the pr=0 output projection into the middle of pr=1
    attention, so ScalarE (exp) and TensorE stay busy together.
    """
    import concourse.bass as bass
    import concourse.mybir as mybir
    import concourse.tile as tile

    fp32 = mybir.dt.float32
    bf16 = mybir.dt.bfloat16
    Exp = mybir.ActivationFunctionType.Exp

    nc = bass.Bass("TRN2", target_bir_lowering=False, debug=False,
                   num_devices=N_CORES)

    xqT = nc.dram_tensor("xqT", [D, S], bf16, kind="ExternalInput")
    xkT = nc.dram_tensor("xkT", [D, S], bf16, kind="ExternalInput")
    xvT = nc.dram_tensor("xvT", [D, S], bf16, kind="ExternalInput")
    wq = nc.dram_tensor("wq", [D, GW], bf16, kind="ExternalInput")
    wk = nc.dram_tensor("wk", [D, GW], bf16, kind="ExternalInput")
    wv = nc.dram_tensor("wv", [D, GW], bf16, kind="ExternalInput")
    wo = nc.dram_tensor("wo", [GW, D], bf16, kind="ExternalInput")
    masks = nc.dram_tensor("masks", [128, 128], bf16, kind="ExternalInput")
    out = nc.dram_tensor("out", [S, D], fp32, kind="ExternalOutput")

    KC = D // 128
    SQT = S // 512
    SKC = S // 128

    with tile.TileContext(nc) as tc:
        with tc.tile_pool(name="wpool", bufs=1) as wpool, \
             tc.tile_pool(name="xpool", bufs=TUNE["xpool"]) as xpool, \
             tc.tile_pool(name="persist", bufs=1) as persist, \
             tc.tile_pool(name="apool", bufs=TUNE["apool"]) as apool, \
             tc.tile_pool(name="rpool", bufs=2) as rpool, \
             tc.tile_pool(name="bpool", bufs=3) as bpool, \
             tc.tile_pool(name="tpool", bufs=2) as tpool, \
             tc.tile_pool(name="opool", bufs=3) as opool, \
             tc.tile_pool(name="pl", bufs=TUNE["pl"], space="PSUM") as pl, \
             tc.tile_pool(name="pc", bufs=1, space="PSUM") as pc:

            wv_sb = wpool.tile([128, KC, GW], bf16, tag="wv")
            wk_sb = wpool.tile([128, KC, GW], bf16, tag="wk")
            wq_sb = wpool.tile([128, KC, GW], bf16, tag="wq")
            wo_sb = wpool.tile([128, GW // 128, D], bf16, tag="wo")
            mask_sb = wpool.tile([128, 128], bf16, tag="masks")
            nc.sync.dma_start(wv_sb[:], wv.ap().rearrange("(kc p) m -> p kc m", p=128))
            nc.sync.dma_start(wk_sb[:], wk.ap().rearrange("(kc p) m -> p kc m", p=128))
            nc.sync.dma_start(wq_sb[:], wq.ap().rearrange("(kc p) m -> p kc m", p=128))
            nc.sync.dma_start(mask_sb[:], masks.ap())
            nc.sync.dma_start(wo_sb[:], wo.ap().rearrange("(m p) d -> p m d", p=128))

            qt = [persist.tile([128, S], bf16, tag=f"qt{m}", name=f"qt{m}")
                  for m in range(4)]
            kt = [persist.tile([128, S], bf16, tag=f"kt{m}", name=f"kt{m}")
                  for m in range(4)]
            vt = [persist.tile([128, HPG * (DK + 1)], bf16, tag=f"vt{j}",
                               name=f"vt{j}") for j in range(SKC)]
            ct = [persist.tile([128, S], bf16, tag=f"ct{m}", name=f"ct{m}")
                  for m in range(4)]

            def load_x(xT):
                xch = []
                for kc in range(KC):
                    xc = xpool.tile([128, S], bf16, tag="xch", name="xch")
                    nc.sync.dma_start(xc[:], xT.ap()[128 * kc:128 * (kc + 1), :])
                    xch.append(xc)
                return xch

            # ---- V projection (natural layout + ones column) -------------
            xch = load_x(xvT)
            for j in range(SKC):
                ps = pl.tile([128, 1024], fp32, tag="pl", name="psv")[:, 0:512]
                for kc in range(KC):
                    nc.tensor.matmul(
                        ps[:], xch[kc][:, 128 * j:128 * (j + 1)], wv_sb[:, kc, :],
                        start=(kc == 0), stop=(kc == KC - 1))
                vt_v = vt[j][:].rearrange("p (h c) -> p h c", c=DK + 1)
                nc.vector.tensor_copy(
                    vt_v[:, :, 0:DK], ps[:].rearrange("p (h c) -> p h c", c=DK))
                nc.vector.memset(vt_v[:, :, DK:DK + 1], 1.0)

            # ---- K projection (transposed layout) ------------------------
            xch = load_x(xkT)
            for m in range(4):
                for js in range(SQT):
                    ps = pl.tile([128, 1024], fp32, tag="pl", name="psk")[:, 0:512]
                    for kc in range(KC):
                        nc.tensor.matmul(
                            ps[:], wk_sb[:, kc, 128 * m:128 * (m + 1)],
                            xch[kc][:, 512 * js:512 * (js + 1)],
                            start=(kc == 0), stop=(kc == KC - 1))
                    nc.vector.tensor_copy(kt[m][:, 512 * js:512 * (js + 1)], ps[:])

            def qproj_js(m, js, xch):
                ps = pl.tile([128, 1024], fp32, tag="pl", name="psq")[:, 0:512]
                for kc in range(KC):
                    nc.tensor.matmul(
                        ps[:], wq_sb[:, kc, 128 * m:128 * (m + 1)],
                        xch[kc][:, 512 * js:512 * (js + 1)],
                        start=(kc == 0), stop=(kc == KC - 1))
                nc.vector.tensor_copy(qt[m][:, 512 * js:512 * (js + 1)], ps[:])

            def qproj(m, xch):
                for js in range(SQT):
                    qproj_js(m, js, xch)

            def attn_steps(pr, h):
                m, po = h // 2, (h % 2) * 64
                qt_h = qt[m][po:po + 64, :]
                kt_h = kt[m][po:po + 64, :]
                nK = 8 * (pr + 1)
                psC = pc.tile([65, 1024], fp32, tag="pc", name="psC")
                for jk in range(nK):
                    off = max(0, 128 * jk - 1024 * pr)
                    kt_sl = kt_h[:, 128 * jk:128 * (jk + 1)]
                    psL = pl.tile([128, 1024], fp32, tag="pl", name="psL")
                    for lo, hi in ((off, 512), (max(off, 512), 1024)):
                        if lo >= hi:
                            continue
                        nc.tensor.matmul(
                            psL[:, lo:hi], kt_sl,
                            qt_h[:, 1024 * pr + lo:1024 * pr + hi],
                            start=True, stop=True)
                    at = apool.tile([128, 1024], bf16, tag="at", name="at")
                    nc.scalar.activation(at[:, off:1024], psL[:, off:1024],
                                         Exp, scale=0.125)
                    if 1024 * pr <= 128 * jk < 1024 * (pr + 1):
                        nc.vector.tensor_mul(at[:, off:off + 128],
                                             at[:, off:off + 128], mask_sb[:])
                    vt_sl = vt[jk][:, (DK + 1) * h:(DK + 1) * (h + 1)]
                    for lo, hi in ((off, 512), (max(off, 512), 1024)):
                        if lo >= hi:
                            continue
                        last = nK - 5 if hi == 512 else nK - 1
                        nc.tensor.matmul(
                            psC[:, lo:hi], vt_sl, at[:, lo:hi],
                            start=(jk == 0), stop=(jk == last))
                    yield
                # release psC quickly (recip + raw copy), then normalize
                # the ct slice in place so pc can stay single-buffered.
                rc = rpool.tile([1, 1024], fp32, tag="rc", name="rc")
                nc.vector.reciprocal(rc[:], psC[64:65, :])
                cs = ct[m][po:po + 64, 1024 * pr:1024 * (pr + 1)]
                if po == 0:
                    nc.vector.tensor_copy(cs, psC[0:64, :])
                else:
                    tmp = tpool.tile([64, 1024], bf16, tag="tmp", name="tmp")
                    nc.vector.tensor_copy(tmp[:], psC[0:64, :])
                    nc.sync.dma_start(cs, tmp[:])
                bc = bpool.tile([128, 1024], fp32, tag="bc", name="bc")
                bch = bc[po:po + 64, :]
                nc.sync.dma_start(
                    bch, rc[0:1, :][:, None, :].to_broadcast((1, 64, 1024)))
                nc.vector.tensor_mul(cs, cs, bch)

            def oproj_steps(pr, i0, i1):
                for i in range(8 * pr + i0, 8 * pr + i1):
                    for n in range(D // 512):
                        psO = pl.tile([128, 1024], fp32, tag="pl", name="psO")[:, 0:512]
                        for m in range(4):
                            nc.tensor.matmul(
                                psO[:], ct[m][:, 128 * i:128 * (i + 1)],
                                wo_sb[:, m, 512 * n:512 * (n + 1)],
                                start=(m == 0), stop=(m == 3))
                        osb = opool.tile([128, 512], fp32, tag="osb", name="osb")
                        nc.vector.tensor_copy(osb[:], psO[:])
                        nc.sync.dma_start(
                            out.ap()[128 * i:128 * (i + 1),
                                     512 * n:512 * (n + 1)], osb[:])
                        yield

            # ---- Q projection interleaved with attention -----------------
            # Attention units are emitted BEFORE the projection slab that
            # runs alongside them: the Tile scheduler picks ready work in
            # priority (emission) order, so the exp-feeding attention
            # matmuls win PE whenever their deps/slots allow, and the
            # projection/output matmuls fill the PE gaps while ScalarE
            # catches up on exp.
            xch = load_x(xqT)
            qproj(0, xch)
            qproj(1, xch)

            def run_interleaved(attn_gens, filler_steps, every):
                n = 0
                fillers = iter(filler_steps)
                for g in attn_gens:
                    for _ in g:
                        n += 1
                        if n % every == 0:
                            f = next(fillers, None)
                            if f is not None:
                                f()
                for f in fillers:
                    f()

            # units 0..3: 32 attention jk-steps each; qproj js-units for
            # m+2 / m+3 dribbled in every 8 steps so ScalarE never starves.
            qfill = {
                0: [lambda js=js: qproj_js(2, js, xch) for js in (0, 1)],
                1: [lambda js=js: qproj_js(2, js, xch) for js in (2, 3)]
                   + [lambda js=js: qproj_js(3, js, xch) for js in (0, 1)],
                2: [lambda js=js: qproj_js(3, js, xch) for js in (2, 3)],
                3: [],
            }
            for m in range(4):
                run_interleaved(
                    [attn_steps(0, 2 * m), attn_steps(0, 2 * m + 1),
                     attn_steps(1, m)],
                    qfill[m], every=TUNE["every"])

            # tail: pr=1 attention for heads 4..7 with oproj(0) dribbled in
            ofill0 = oproj_steps(0, 0, 8)
            run_interleaved(
                [attn_steps(1, 4), attn_steps(1, 5), attn_steps(1, 6),
                 attn_steps(1, 7)],
                [lambda: next(ofill0, None) for _ in range(16)], every=4)
            for _ in oproj_steps(1, 0, 8):
                pass
    return nc


def make_masks():
    """Diagonal triangle mask [sk_r, sq_c]: keep (1.0) where c >= r."""
    r = np.arange(128)[:, None]
    c = np.arange(128)[None, :]
    return (c >= r).astype(BF16)


def make_in_maps(queries, keys, values, Wq, Wk, Wv, Wo):
    masks = make_masks()
    # per-batch transposed bf16 activations, shared by both head-group cores
    xT = {}
    for b in range(B):
        xT[b] = tuple(
            np.ascontiguousarray(np.asarray(x)[b].astype(BF16).T)
            for x in (queries, keys, values))
    wg = {}
    for g in range(2):
        sl = slice(g * GW, (g + 1) * GW)
        wg[g] = (np.asarray(Wq)[:, sl].astype(BF16),
                 np.asarray(Wk)[:, sl].astype(BF16),
                 np.asarray(Wv)[:, sl].astype(BF16),
                 np.ascontiguousarray(np.asarray(Wo)[sl, :]).astype(BF16))
    in_maps = []
    for c in range(N_CORES):
        b, g = c // 2, c % 2
        q, k, v = xT[b]
        wq_, wk_, wv_, wo_ = wg[g]
        in_maps.append({"xqT": q, "xkT": k, "xvT": v, "wq": wq_, "wk": wk_,
                        "wv": wv_, "wo": wo_, "masks": masks})
    return in_maps


def kernel(queries, keys, values, mask, Wq, Wk, Wv, Wo, bo):
    _apply_compat_patches()
    from concourse.bass_utils import run_bass_kernel_spmd

    key = "attn"
    if key not in _nc_cache:
        _nc_cache[key] = build_attention_nc()
    nc = _nc_cache[key]

    in_maps = make_in_maps(queries, keys, values, Wq, Wk, Wv, Wo)
    res = run_bass_kernel_spmd(nc, in_maps, core_ids=list(range(N_CORES)))

    out = np.empty((B, S, D), dtype=np.float32)
    for b in range(B):
        out[b] = res.results[2 * b]["out"] + res.results[2 * b + 1]["out"]
    out += bo.astype(np.float32)[None, None, :]
    return out


# revision 27
# speedup vs baseline: 1.0329x; 1.0329x over previous
"""Multi-head causal attention (B=4, S=2048, D=1024, H=16) on 8 NeuronCores.

Sharding: batch x head-group. Core c handles batch b = c//2 and head group
g = c%2 (8 heads of 64 dims each). Wq/Wk/Wv are column-split per head group
(Megatron column-parallel), Wo is row-split; each core returns a partial
output [S, D] which the host sums over the two head-group cores per batch.

Device kernel (identical SPMD program on all 8 cores, bf16 compute,
fp32 accumulation):
  1. QT/KT = (X @ Wg)^T computed directly in transposed layout
     (dk on partitions) so attention matmuls need no on-device transpose.
     V computed in natural layout [sk, dv] and packed with a ones-column
     per head (denominator trick).
  2. Per head: L^T tiles [sk=128, sq=512] = KT_h^T-slices @ QT_h (K=64),
     exp via ScalarE with the 1/sqrt(dk) folded into the activation scale,
     causal masking on diagonal tiles via a preloaded 0/1 mask multiply,
     then C~^T[65, sq] += Vtilde^T @ A^T accumulated over sk chunks: rows
     0..63 are the unnormalized context^T, row 64 the softmax denominator.
     Normalize with reciprocal + DMA row-broadcast + VectorE multiply.
  3. Output projection: ct pair tiles [128(dv), 2048(sq)] @ Wo rows,
     accumulating all 8 heads (K=128 per matmul), PSUM -> DRAM f32.
"""
import json

import numpy as np
import ml_dtypes

BF16 = ml_dtypes.bfloat16

B, S, D = 4, 2048, 1024
H = 16
DK = 64          # per-head dim
HPG = 8          # heads per group
GW = HPG * DK    # group width = 512
N_CORES = 8

_nc_cache = {}
TUNE = {"apool": 8, "xpool": 12, "every": 8, "pl": 3}


def _apply_compat_patches():
    """This container's walrus rejects instructions carrying more than one
    sem-wait ("Too many sync wait commands"). Split excess waits onto NoOps
    on the same engine, patched into every compile path."""
    import concourse.bass_utils as bass_utils

    if getattr(bass_utils, "_wait_split_patched", False):
        return
    _orig = bass_utils.compile_bir_kernel
    seq = [0]

    def split_bir_waits(bir, limit=1):
        for fn in bir.get("functions", []):
            for bb in fn.get("blocks", []):
                out, changed = [], False
                for ins in bb.get("instructions", []):
                    si = ins.get("sync_info")
                    ow = (si or {}).get("on_wait") or []
                    if len(ow) > limit:
                        changed = True
                        extra, keep = ow[:-limit], ow[-limit:]
                        for i in range(0, len(extra), limit):
                            seq[0] += 1
                            out.append({
                                "debug": ins.get("debug", 0),
                                "engine": ins["engine"],
                                "ins": [], "outs": [],
                                "name": f"WSPLIT-{seq[0]}",
                                "opcode": "NoOp",
                                "sync_info": {"on_update": [],
                                              "on_wait": extra[i:i + limit]},
                            })
                        si["on_wait"] = keep
                    out.append(ins)
                if changed:
                    bb["instructions"] = out
        return bir

    def _patched(bir_json, tmpdir, neff_name="file.neff", **kw):
        bir = split_bir_waits(json.loads(bir_json))
        return _orig(json.dumps(bir).encode(), tmpdir, neff_name, **kw)

    bass_utils.compile_bir_kernel = _patched
    bass_utils._wait_split_patched = True
    try:
        import concourse.bass2jax as bass2jax
        bass2jax.compile_bir_kernel = _patched
    except Exception:
        pass


def build_attention_nc():
    """Build the SPMD Bass program (one NeuronCore's view).

    Emission order interleaves Q-projection with pr=0 attention per head
    pair, and slots the pr=0 output projection into the middle of pr=1
    attention, so ScalarE (exp) and TensorE stay busy together.
    """
    import concourse.bass as bass
    import concourse.mybir as mybir
    import concourse.tile as tile

    fp32 = mybir.dt.float32
    bf16 = mybir.dt.bfloat16
    Exp = mybir.ActivationFunctionType.Exp

    nc = bass.Bass("TRN2", target_bir_lowering=False, debug=False,
                   num_devices=N_CORES)

    xqT = nc.dram_tensor("xqT", [D, S], bf16, kind="ExternalInput")
    xkT = nc.dram_tensor("xkT", [D, S], bf16, kind="ExternalInput")
    xvT = nc.dram_tensor("xvT", [D, S], bf16, kind="ExternalInput")
    wq = nc.dram_tensor("wq", [D, GW], bf16, kind="ExternalInput")
    wk = nc.dram_tensor("wk", [D, GW], bf16, kind="ExternalInput")
    wv = nc.dram_tensor("wv", [D, GW], bf16, kind="ExternalInput")
    wo = nc.dram_tensor("wo", [GW, D], bf16, kind="ExternalInput")
    masks = nc.dram_tensor("masks", [128, 128], bf16, kind="ExternalInput")
    out = nc.dram_tensor("out", [S, D], fp32, kind="ExternalOutput")

    KC = D // 128
    SQT = S // 512
    SKC = S // 128

    with tile.TileContext(nc) as tc:
        with tc.tile_pool(name="wpool", bufs=1) as wpool, \
             tc.tile_pool(name="xpool", bufs=TUNE["xpool"]) as xpool, \
             tc.tile_pool(name="persist", bufs=1) as persist, \
             tc.tile_pool(name="apool", bufs=TUNE["apool"]) as apool, \
             tc.tile_pool(name="rpool", bufs=2) as rpool, \
             tc.tile_pool(name="bpool", bufs=3) as bpool, \
             tc.tile_pool(name="tpool", bufs=2) as tpool, \
             tc.tile_pool(name="opool", bufs=3) as opool, \
             tc.tile_pool(name="pl", bufs=TUNE["pl"], space="PSUM") as pl, \
             tc.tile_pool(name="pc", bufs=1, space="PSUM") as pc:

            wv_sb = wpool.tile([128, KC, GW], bf16, tag="wv")
            wk_sb = wpool.tile([128, KC, GW], bf16, tag="wk")
            wq_sb = wpool.tile([128, KC, GW], bf16, tag="wq")
            wo_sb = wpool.tile([128, GW // 128, D], bf16, tag="wo")
            mask_sb = wpool.tile([128, 128], bf16, tag="masks")
            nc.sync.dma_start(wv_sb[:], wv.ap().rearrange("(kc p) m -> p kc m", p=128))

            qt = [persist.tile([128, S], bf16, tag=f"qt{m}", name=f"qt{m}")
                  for m in range(4)]
            kt = [persist.tile([128, S], bf16, tag=f"kt{m}", name=f"kt{m}")
                  for m in range(4)]
            vt = [persist.tile([128, HPG * (DK + 1)], bf16, tag=f"vt{j}",
                               name=f"vt{j}") for j in range(SKC)]
            ct = [persist.tile([128, S], bf16, tag=f"ct{m}", name=f"ct{m}")
                  for m in range(4)]

            def load_x(xT):
                xch = []
                for kc in range(KC):
                    xc = xpool.tile([128, S], bf16, tag="xch", name="xch")
                    nc.sync.dma_start(xc[:], xT.ap()[128 * kc:128 * (kc + 1), :])
                    xch.append(xc)
                return xch

            # ---- V projection (natural layout + ones column) -------------
            xch = load_x(xvT)
            for j in range(SKC):
                ps = pl.tile([128, 1024], fp32, tag="pl", name="psv")[:, 0:512]
                for kc in range(KC):
                    nc.tensor.matmul(
                        ps[:], xch[kc][:, 128 * j:128 * (j + 1)], wv_sb[:, kc, :],
                        start=(kc == 0), stop=(kc == KC - 1))
                vt_v = vt[j][:].rearrange("p (h c) -> p h c", c=DK + 1)
                nc.vector.tensor_copy(
                    vt_v[:, :, 0:DK], ps[:].rearrange("p (h c) -> p h c", c=DK))
                nc.vector.memset(vt_v[:, :, DK:DK + 1], 1.0)

            # ---- K projection (transposed layout) ------------------------
            nc.sync.dma_start(wk_sb[:], wk.ap().rearrange("(kc p) m -> p kc m", p=128))
            xch = load_x(xkT)
            for m in range(4):
                for js in range(SQT):
                    ps = pl.tile([128, 1024], fp32, tag="pl", name="psk")[:, 0:512]
                    for kc in range(KC):
                        nc.tensor.matmul(
                            ps[:], wk_sb[:, kc, 128 * m:128 * (m + 1)],
                            xch[kc][:, 512 * js:512 * (js + 1)],
                            start=(kc == 0), stop=(kc == KC - 1))
                    nc.vector.tensor_copy(kt[m][:, 512 * js:512 * (js + 1)], ps[:])

            def qproj_js(m, js, xch):
                ps = pl.tile([128, 1024], fp32, tag="pl", name="psq")[:, 0:512]
                for kc in range(KC):
                    nc.tensor.matmul(
                        ps[:], wq_sb[:, kc, 128 * m:128 * (m + 1)],
                        xch[kc][:, 512 * js:512 * (js + 1)],
                        start=(kc == 0), stop=(kc == KC - 1))
                nc.vector.tensor_copy(qt[m][:, 512 * js:512 * (js + 1)], ps[:])

            def qproj(m, xch):
                for js in range(SQT):
                    qproj_js(m, js, xch)

            def attn_steps(pr, h):
                m, po = h // 2, (h % 2) * 64
                qt_h = qt[m][po:po + 64, :]
                kt_h = kt[m][po:po + 64, :]
                nK = 8 * (pr + 1)
                psC = pc.tile([65, 1024], fp32, tag="pc", name="psC")
                for jk in range(nK):
                    off = max(0, 128 * jk - 1024 * pr)
                    kt_sl = kt_h[:, 128 * jk:128 * (jk + 1)]
                    psL = pl.tile([128, 1024], fp32, tag="pl", name="psL")
                    for lo, hi in ((off, 512), (max(off, 512), 1024)):
                        if lo >= hi:
                            continue
                        nc.tensor.matmul(
                            psL[:, lo:hi], kt_sl,
                            qt_h[:, 1024 * pr + lo:1024 * pr + hi],
                            start=True, stop=True)
                    at = apool.tile([128, 1024], bf16, tag="at", name="at")
                    nc.scalar.activation(at[:, off:1024], psL[:, off:1024],
                                         Exp, scale=0.125)
                    if 1024 * pr <= 128 * jk < 1024 * (pr + 1):
                        nc.vector.tensor_mul(at[:, off:off + 128],
                                             at[:, off:off + 128], mask_sb[:])
                    vt_sl = vt[jk][:, (DK + 1) * h:(DK + 1) * (h + 1)]
                    for lo, hi in ((off, 512), (max(off, 512), 1024)):
                        if lo >= hi:
                            continue
                        last = nK - 5 if hi == 512 else nK - 1
                        nc.tensor.matmul(
                            psC[:, lo:hi], vt_sl, at[:, lo:hi],
                            start=(jk == 0), stop=(jk == last))
                    yield
                # release psC quickly (recip + raw copy), then normalize
                # the ct slice in place so pc can stay single-buffered.
                rc = rpool.tile([1, 1024], fp32, tag="rc", name="rc")
                nc.vector.reciprocal(rc[:], psC[64:65, :])
                cs = ct[m][po:po + 64, 1024 * pr:1024 * (pr + 1)]
                if po == 0:
                    nc.vector.tensor_copy(cs, psC[0:64, :])
                else:
                    tmp = tpool.tile([64, 1024], bf16, tag="tmp", name="tmp")
                    nc.vector.tensor_copy(tmp[:], psC[0:64, :])
                    nc.sync.dma_start(cs, tmp[:])
                bc = bpool.tile([128, 1024], fp32, tag="bc", name="bc")
                bch = bc[po:po + 64, :]
                nc.sync.dma_start(
                    bch, rc[0:1, :][:, None, :].to_broadcast((1, 64, 1024)))
                nc.vector.tensor_mul(cs, cs, bch)

            def oproj_steps(pr, i0, i1):
                for i in range(8 * pr + i0, 8 * pr + i1):
                    for n in range(D // 512):
                        psO = pl.tile([128, 1024], fp32, tag="pl", name="psO")[:, 0:512]
                        for m in range(4):
                            nc.tensor.matmul(
                                psO[:], ct[m][:, 128 * i:128 * (i + 1)],
                                wo_sb[:, m, 512 * n:512 * (n + 1)],
                                start=(m == 0), stop=(m == 3))
                        osb = opool.tile([128, 512], fp32, tag="osb", name="osb")
                        nc.vector.tensor_copy(osb[:], psO[:])
                        nc.sync.dma_start(
                            out.ap()[128 * i:128 * (i + 1),
                                     512 * n:512 * (n + 1)], osb[:])
                        yield

            # ---- Q projection interleaved with attention -----------------
            # Attention units are emitted BEFORE the projection slab that
            # runs alongside them: the Tile scheduler picks ready work in
            # priority (emission) order, so the exp-feeding attention
            # matmuls win PE whenever their deps/slots allow, and the
            # projection/output matmuls fill the PE gaps while ScalarE
            # catches up on exp.
            nc.sync.dma_start(wq_sb[:], wq.ap().rearrange("(kc p) m -> p kc m", p=128))
            nc.sync.dma_start(mask_sb[:], masks.ap())
            nc.sync.dma_start(wo_sb[:], wo.ap().rearrange("(m p) d -> p m d", p=128))
            xch = load_x(xqT)
            qproj(0, xch)
            qproj(1, xch)

            def run_interleaved(attn_gens, filler_steps, every):
                n = 0
                fillers = iter(filler_steps)
                for g in attn_gens:
                    for _ in g:
                        n += 1
                        if n % every == 0:
                            f = next(fillers, None)
                            if f is not None:
                                f()
                for f in fillers:
                    f()

            # units 0..3: 32 attention jk-steps each; qproj js-units for
            # m+2 / m+3 dribbled in every 8 steps so ScalarE never starves.
            qfill = {
                0: [lambda js=js: qproj_js(2, js, xch) for js in (0, 1)],
                1: [lambda js=js: qproj_js(2, js, xch) for js in (2, 3)]
                   + [lambda js=js: qproj_js(3, js, xch) for js in (0, 1)],
                2: [lambda js=js: qproj_js(3, js, xch) for js in (2, 3)],
                3: [],
            }
            for m in range(4):
                run_interleaved(
                    [attn_steps(0, 2 * m), attn_steps(0, 2 * m + 1),
                     attn_steps(1, m)],
                    qfill[m], every=TUNE["every"])

            # tail: pr=1 attention for heads 4..7 with oproj(0) dribbled in
            ofill0 = oproj_steps(0, 0, 8)
            run_interleaved(
                [attn_steps(1, 4), attn_steps(1, 5), attn_steps(1, 7),
                 attn_steps(1, 6)],
                [lambda: next(ofill0, None) for _ in range(16)], every=4)
            for _ in oproj_steps(1, 0, 8):
                pass
    return nc


def make_masks():
    """Diagonal triangle mask [sk_r, sq_c]: keep (1.0) where c >= r."""
    r = np.arange(128)[:, None]
    c = np.arange(128)[None, :]
    return (c >= r).astype(BF16)


def make_in_maps(queries, keys, values, Wq, Wk, Wv, Wo):
    masks = make_masks()
    # per-batch transposed bf16 activations, shared by both head-group cores
    xT = {}
    for b in range(B):
        xT[b] = tuple(
            np.ascontiguousarray(np.asarray(x)[b].astype(BF16).T)
            for x in (queries, keys, values))
    wg = {}
    for g in range(2):
        sl = slice(g * GW, (g + 1) * GW)
        wg[g] = (np.asarray(Wq)[:, sl].astype(BF16),
                 np.asarray(Wk)[:, sl].astype(BF16),
                 np.asarray(Wv)[:, sl].astype(BF16),
                 np.ascontiguousarray(np.asarray(Wo)[sl, :]).astype(BF16))
    in_maps = []
    for c in range(N_CORES):
        b, g = c // 2, c % 2
        q, k, v = xT[b]
        wq_, wk_, wv_, wo_ = wg[g]
        in_maps.append({"xqT": q, "xkT": k, "xvT": v, "wq": wq_, "wk": wk_,
                        "wv": wv_, "wo": wo_, "masks": masks})
    return in_maps


def kernel(queries, keys, values, mask, Wq, Wk, Wv, Wo, bo):
    _apply_compat_patches()
    from concourse.bass_utils import run_bass_kernel_spmd

    key = "attn"
    if key not in _nc_cache:
        _nc_cache[key] = build_attention_nc()
    nc = _nc_cache[key]

    in_maps = make_in_maps(queries, keys, values, Wq, Wk, Wv, Wo)
    res = run_bass_kernel_spmd(nc, in_maps, core_ids=list(range(N_CORES)))

    out = np.empty((B, S, D), dtype=np.float32)
    for b in range(B):
        out[b] = res.results[2 * b]["out"] + res.results[2 * b + 1]["out"]
    out += bo.astype(np.float32)[None, None, :]
    return out
